# revision 8
# baseline (speedup 1.0000x reference)
"""Batched normalized-gram kernel for 8 TRN2 NeuronCores.

reference:  x (64, 2, 512, 512) fp32
    x0 = x[:, 0]                               (B=64, V=512, F=512)
    n  = sqrt(sum(x0^2, axis=(0, 2)))          (V,)
    out[b] = (x0[b] @ x0[b].T) / outer(n, n)   (B, V, V)

Since gram[b,i,j]/(n_i n_j) == (x0[b,i,:]/n_i) . (x0[b,j,:]/n_j), the host
prescales rows by 1/n once and the device work is a pure batched symmetric
matmul out[b] = y[b] @ y[b].T.

Device-side tricks:
  * operands shipped as fp16 (|y| <= ~0.05, comfortably normal) — halves
    input DMA, full-rate PE, fp32 PSUM accumulation keeps rel err ~2e-4.
  * out[b] is symmetric, and the reference is *exactly* symmetric (same
    products, same summation order), so the device computes only the upper
    block-triangle (row-block mi covers columns mi*128..511) and the host
    mirrors the lower blocks.  -37.5% output DMA, -37.5% PE work.
  * output shipped as a packed fp16 [128, 1280] tile per batch (one DMA,
    2560B contiguous rows) — halves output HBM traffic again and cuts
    descriptor-generation work 4x; host unpacks + upcasts.
  * input shipped pair-of-batches interleaved so each DMA row is 2KB
    (better SDMA descriptor efficiency than 1KB rows).

Sharding: data-parallel over batch — 8 batches per core, no collectives.
"""

import numpy as np

B, T, V, F = 64, 2, 512, 512
NCORES = 8
BPC = B // NCORES  # batches per core
NBLK = V // 128  # 4 row-blocks
NG = BPC // 2  # pair-groups per core

# upper-triangle segment offsets inside the per-batch packed output tile:
# row-block mi holds columns mi*128..511 (n_cols = 512 - 128*mi)
SEG_OFF = [0]
for _mi in range(NBLK):
    SEG_OFF.append(SEG_OFF[-1] + V - 128 * _mi)
SEG_TOTAL = SEG_OFF[-1]  # 1280

_NC = None


def _build_nc():
    import concourse.mybir as mybir
    import concourse.tile as tile
    from concourse import bacc

    f32 = mybir.dt.float32
    f16 = mybir.dt.float16

    nc = bacc.Bacc(target_bir_lowering=False)
    # yP[g, f, b*V + v] = y[2g+b, v, f]  — 2KB rows per partition line
    yP = nc.declare_dram_parameter("yP", [NG, F, 2 * V], f16, isOutput=False)
    outP = nc.declare_dram_parameter(
        "outP", [BPC, 128, SEG_TOTAL], f16, isOutput=True
    )

    with tile.TileContext(nc) as tc:
        with (
            tc.tile_pool(name="sb", bufs=24) as sb_pool,
            tc.tile_pool(name="psum", bufs=8, space="PSUM") as psum_pool,
        ):
            # PE warmup: the HAM clock gate keeps the PE at 1.2 GHz until it
            # has seen ~3.4us of sustained activity.  Burn that window on
            # garbage matmuls while the first input chunks stream in so the
            # real matmuls run at 2.4 GHz from the start.
            warm = sb_pool.tile([128, 640], f16, tag="warm")
            nc.gpsimd.memset(warm, 0)
            wps = psum_pool.tile([128, 512], f32, tag="ps")
            for _ in range(4):
                nc.tensor.matmul(
                    wps,
                    lhsT=warm[:, 512:640],
                    rhs=warm[:, 0:512],
                    start=True,
                    stop=True,
                )
            for g in range(NG):
                # 4 partition-chunks of [128, 2V]: two batches side by side.
                # For the first group, alternate chunks across the two HWDGE
                # rings (SP + ACT) so chunk0 drains alone on its ring and the
                # PE can start ~2us earlier; later groups all ride SP (the
                # ACT ring carries output DMAs by then).
                chunks = []
                for ki in range(NBLK):
                    ck = sb_pool.tile([128, 2 * V], f16, tag="ck")
                    eng = nc.scalar if (g == 0 and ki % 2 == 1) else nc.sync
                    eng.dma_start(
                        out=ck, in_=yP[g, ki * 128 : (ki + 1) * 128, :]
                    )
                    chunks.append(ck)
                for b in range(2):
                    bb = 2 * g + b
                    base = b * V
                    ot = sb_pool.tile([128, SEG_TOTAL], f16, tag="ot")
                    pss = [
                        psum_pool.tile(
                            [128, V - 128 * mi], f32, tag="ps",
                            name=f"ps_{bb}_{mi}",
                        )
                        for mi in range(NBLK)
                    ]
                    if g == 0:
                        # ki-outer: accumulate all 4 row-blocks per input
                        # chunk so the PE starts as soon as chunk0 lands
                        # instead of waiting for the whole group.
                        order = [
                            (ki, mi)
                            for ki in range(NBLK)
                            for mi in range(NBLK)
                        ]
                    else:
                        # mi-outer: each row-block finishes (and casts/ships)
                        # while the next one computes — keeps the tail short.
                        order = [
                            (ki, mi)
                            for mi in range(NBLK)
                            for ki in range(NBLK)
                        ]
                    for ki, mi in order:
                        nc.tensor.matmul(
                            pss[mi],
                            lhsT=chunks[ki][
                                :, base + mi * 128 : base + (mi + 1) * 128
                            ],
                            rhs=chunks[ki][:, base + mi * 128 : base + V],
                            start=(ki == 0),
                            stop=(ki == NBLK - 1),
                        )
                    for mi in range(NBLK):
                        nc.vector.tensor_copy(
                            out=ot[:, SEG_OFF[mi] : SEG_OFF[mi + 1]],
                            in_=pss[mi],
                        )
                    # split the output DMA so earlier blocks ship while later
                    # ones are still being cast; the last batch ships finer,
                    # with its final sliver on the idle SP ring so the two
                    # last triggers run concurrently.
                    if bb == BPC - 1:
                        nc.scalar.dma_start(
                            out=outP[bb][:, 0 : SEG_OFF[2]],
                            in_=ot[:, 0 : SEG_OFF[2]],
                        )
                        nc.scalar.dma_start(
                            out=outP[bb][:, SEG_OFF[2] : SEG_OFF[3]],
                            in_=ot[:, SEG_OFF[2] : SEG_OFF[3]],
                        )
                        nc.sync.dma_start(
                            out=outP[bb][:, SEG_OFF[3] :],
                            in_=ot[:, SEG_OFF[3] :],
                        )
                    else:
                        nc.scalar.dma_start(
                            out=outP[bb][:, 0 : SEG_OFF[2]],
                            in_=ot[:, 0 : SEG_OFF[2]],
                        )
                        nc.scalar.dma_start(
                            out=outP[bb][:, SEG_OFF[2] :],
                            in_=ot[:, SEG_OFF[2] :],
                        )
    if not nc.is_finalized():
        nc.finalize()
    return nc


def _get_nc():
    global _NC
    if _NC is None:
        _NC = _build_nc()
    return _NC


def _prep_shards(x: np.ndarray) -> np.ndarray:
    x = np.ascontiguousarray(np.asarray(x, dtype=np.float32))
    x0 = x[:, 0]  # (B, V, F)
    ss = np.einsum("bvf,bvf->v", x0, x0, optimize=True)
    inv_n = (1.0 / np.sqrt(ss)).astype(np.float32)
    y = x0 * inv_n[None, :, None]
    # (NCORES, NG, F, 2V): per core, pair-groups of batches, F-major so each
    # DMA partition row is a contiguous 2KB line covering both batches.
    yP = np.transpose(
        y.reshape(NCORES, NG, 2, V, F), (0, 1, 4, 2, 3)
    ).reshape(NCORES, NG, F, 2 * V)
    return np.ascontiguousarray(yP.astype(np.float16))


def kernel(x: np.ndarray, _trace: bool = False, _trace_out: list | None = None):
    from concourse.bass_utils import run_bass_kernel_spmd

    yP = _prep_shards(x)
    nc = _get_nc()
    in_maps = [{"yP": yP[c]} for c in range(NCORES)]
    res = run_bass_kernel_spmd(
        nc, in_maps, core_ids=list(range(NCORES)), trace=_trace
    )
    if _trace_out is not None:
        _trace_out.append(res)
    packed = np.concatenate(
        [np.asarray(res.results[c]["outP"]) for c in range(NCORES)], axis=0
    )  # (B, 128, 1280) fp16
    full = np.empty((B, V, V), dtype=np.float32)
    for mi in range(NBLK):
        n_cols = V - 128 * mi
        full[:, mi * 128 : (mi + 1) * 128, mi * 128 :] = packed[
            :, :, SEG_OFF[mi] : SEG_OFF[mi] + n_cols
        ].astype(np.float32)
    # device wrote only the upper block-triangle; mirror it down
    for mi in range(NBLK):
        for nj in range(mi + 1, NBLK):
            full[:, nj * 128 : (nj + 1) * 128, mi * 128 : (mi + 1) * 128] = (
                np.swapaxes(
                    full[:, mi * 128 : (mi + 1) * 128, nj * 128 : (nj + 1) * 128],
                    1,
                    2,
                )
            )
    return full


# revision 11
# speedup vs baseline: 1.0060x; 1.0060x over previous
"""Batched normalized-gram kernel for 8 TRN2 NeuronCores.

reference:  x (64, 2, 512, 512) fp32
    x0 = x[:, 0]                               (B=64, V=512, F=512)
    n  = sqrt(sum(x0^2, axis=(0, 2)))          (V,)
    out[b] = (x0[b] @ x0[b].T) / outer(n, n)   (B, V, V)

Since gram[b,i,j]/(n_i n_j) == (x0[b,i,:]/n_i) . (x0[b,j,:]/n_j), the host
prescales rows by 1/n once and the device work is a pure batched symmetric
matmul out[b] = y[b] @ y[b].T.

Device-side tricks:
  * operands shipped as fp16 (|y| <= ~0.05, comfortably normal) — halves
    input DMA, full-rate PE, fp32 PSUM accumulation keeps rel err ~2e-4.
  * out[b] is symmetric, and the reference is *exactly* symmetric (same
    products, same summation order), so the device computes only the upper
    block-triangle (row-block mi covers columns mi*128..511) and the host
    mirrors the lower blocks.  -37.5% output DMA, -37.5% PE work.
  * output shipped as a packed fp16 [128, 1280] tile per batch (one DMA,
    2560B contiguous rows) — halves output HBM traffic again and cuts
    descriptor-generation work 4x; host unpacks + upcasts.
  * input shipped pair-of-batches interleaved so each DMA row is 2KB
    (better SDMA descriptor efficiency than 1KB rows).

Sharding: data-parallel over batch — 8 batches per core, no collectives.
"""

import numpy as np

B, T, V, F = 64, 2, 512, 512
NCORES = 8
BPC = B // NCORES  # batches per core
NBLK = V // 128  # 4 row-blocks
NG = BPC // 2  # pair-groups per core

# upper-triangle segment offsets inside the per-batch packed output tile:
# row-block mi holds columns mi*128..511 (n_cols = 512 - 128*mi)
SEG_OFF = [0]
for _mi in range(NBLK):
    SEG_OFF.append(SEG_OFF[-1] + V - 128 * _mi)
SEG_TOTAL = SEG_OFF[-1]  # 1280

_NC = None


def _build_nc():
    import concourse.mybir as mybir
    import concourse.tile as tile
    from concourse import bacc

    f32 = mybir.dt.float32
    f16 = mybir.dt.float16

    nc = bacc.Bacc(target_bir_lowering=False)
    # yP[g, f, b*V + v] = y[2g+b, v, f]  — 2KB rows per partition line
    yP = nc.declare_dram_parameter("yP", [NG, F, 2 * V], f16, isOutput=False)
    outP = nc.declare_dram_parameter(
        "outP", [BPC, 128, SEG_TOTAL], f16, isOutput=True
    )

    with tile.TileContext(nc) as tc:
        with (
            tc.tile_pool(name="sb", bufs=24) as sb_pool,
            tc.tile_pool(name="psum", bufs=8, space="PSUM") as psum_pool,
        ):
            # PE warmup: the HAM clock gate keeps the PE at 1.2 GHz until it
            # has seen ~3.4us of sustained activity.  Burn that window on
            # garbage matmuls while the first input chunks stream in so the
            # real matmuls run at 2.4 GHz from the start.
            warm = sb_pool.tile([128, 640], f16, tag="warm")
            nc.gpsimd.memset(warm, 0)
            wps = psum_pool.tile([128, 512], f32, tag="ps")
            for _ in range(6):
                nc.tensor.matmul(
                    wps,
                    lhsT=warm[:, 512:640],
                    rhs=warm[:, 0:512],
                    start=True,
                    stop=True,
                )
            for g in range(NG):
                # 4 partition-chunks of [128, 2V]: two batches side by side.
                # For the first group, alternate chunks across the two HWDGE
                # rings (SP + ACT) so chunk0 drains alone on its ring and the
                # PE can start ~2us earlier; later groups all ride SP (the
                # ACT ring carries output DMAs by then).
                chunks = []
                for ki in range(NBLK):
                    ck = sb_pool.tile([128, 2 * V], f16, tag="ck")
                    eng = nc.scalar if (g == 0 and ki % 2 == 1) else nc.sync
                    eng.dma_start(
                        out=ck, in_=yP[g, ki * 128 : (ki + 1) * 128, :]
                    )
                    chunks.append(ck)
                for b in range(2):
                    bb = 2 * g + b
                    base = b * V
                    ot = sb_pool.tile([128, SEG_TOTAL], f16, tag="ot")
                    pss = [
                        psum_pool.tile(
                            [128, V - 128 * mi], f32, tag="ps",
                            name=f"ps_{bb}_{mi}",
                        )
                        for mi in range(NBLK)
                    ]
                    if g == 0:
                        # ki-outer: accumulate all 4 row-blocks per input
                        # chunk so the PE starts as soon as chunk0 lands
                        # instead of waiting for the whole group.
                        order = [
                            (ki, mi)
                            for ki in range(NBLK)
                            for mi in range(NBLK)
                        ]
                    else:
                        # mi-outer: each row-block finishes (and casts/ships)
                        # while the next one computes — keeps the tail short.
                        order = [
                            (ki, mi)
                            for mi in range(NBLK)
                            for ki in range(NBLK)
                        ]
                    for ki, mi in order:
                        nc.tensor.matmul(
                            pss[mi],
                            lhsT=chunks[ki][
                                :, base + mi * 128 : base + (mi + 1) * 128
                            ],
                            rhs=chunks[ki][:, base + mi * 128 : base + V],
                            start=(ki == 0),
                            stop=(ki == NBLK - 1),
                        )
                    for mi in range(NBLK):
                        nc.vector.tensor_copy(
                            out=ot[:, SEG_OFF[mi] : SEG_OFF[mi + 1]],
                            in_=pss[mi],
                        )
                    # split the output DMA so earlier blocks ship while later
                    # ones are still being cast; the last batch ships finer,
                    # with its final sliver on the idle SP ring so the two
                    # last triggers run concurrently.
                    if bb == BPC - 1:
                        nc.scalar.dma_start(
                            out=outP[bb][:, 0 : SEG_OFF[2]],
                            in_=ot[:, 0 : SEG_OFF[2]],
                        )
                        nc.scalar.dma_start(
                            out=outP[bb][:, SEG_OFF[2] : SEG_OFF[3]],
                            in_=ot[:, SEG_OFF[2] : SEG_OFF[3]],
                        )
                        nc.sync.dma_start(
                            out=outP[bb][:, SEG_OFF[3] :],
                            in_=ot[:, SEG_OFF[3] :],
                        )
                    else:
                        nc.scalar.dma_start(
                            out=outP[bb][:, 0 : SEG_OFF[2]],
                            in_=ot[:, 0 : SEG_OFF[2]],
                        )
                        nc.scalar.dma_start(
                            out=outP[bb][:, SEG_OFF[2] :],
                            in_=ot[:, SEG_OFF[2] :],
                        )
    if not nc.is_finalized():
        nc.finalize()
    return nc


def _get_nc():
    global _NC
    if _NC is None:
        _NC = _build_nc()
    return _NC


def _prep_shards(x: np.ndarray) -> np.ndarray:
    x = np.ascontiguousarray(np.asarray(x, dtype=np.float32))
    x0 = x[:, 0]  # (B, V, F)
    ss = np.einsum("bvf,bvf->v", x0, x0, optimize=True)
    inv_n = (1.0 / np.sqrt(ss)).astype(np.float32)
    y = x0 * inv_n[None, :, None]
    # (NCORES, NG, F, 2V): per core, pair-groups of batches, F-major so each
    # DMA partition row is a contiguous 2KB line covering both batches.
    yP = np.transpose(
        y.reshape(NCORES, NG, 2, V, F), (0, 1, 4, 2, 3)
    ).reshape(NCORES, NG, F, 2 * V)
    return np.ascontiguousarray(yP.astype(np.float16))


def kernel(x: np.ndarray, _trace: bool = False, _trace_out: list | None = None):
    from concourse.bass_utils import run_bass_kernel_spmd

    yP = _prep_shards(x)
    nc = _get_nc()
    in_maps = [{"yP": yP[c]} for c in range(NCORES)]
    res = run_bass_kernel_spmd(
        nc, in_maps, core_ids=list(range(NCORES)), trace=_trace
    )
    if _trace_out is not None:
        _trace_out.append(res)
    packed = np.concatenate(
        [np.asarray(res.results[c]["outP"]) for c in range(NCORES)], axis=0
    )  # (B, 128, 1280) fp16
    full = np.empty((B, V, V), dtype=np.float32)
    for mi in range(NBLK):
        n_cols = V - 128 * mi
        full[:, mi * 128 : (mi + 1) * 128, mi * 128 :] = packed[
            :, :, SEG_OFF[mi] : SEG_OFF[mi] + n_cols
        ].astype(np.float32)
    # device wrote only the upper block-triangle; mirror it down
    for mi in range(NBLK):
        for nj in range(mi + 1, NBLK):
            full[:, nj * 128 : (nj + 1) * 128, mi * 128 : (mi + 1) * 128] = (
                np.swapaxes(
                    full[:, mi * 128 : (mi + 1) * 128, nj * 128 : (nj + 1) * 128],
                    1,
                    2,
                )
            )
    return full


# revision 13
# speedup vs baseline: 1.0066x; 1.0005x over previous
"""Batched normalized-gram kernel for 8 TRN2 NeuronCores.

reference:  x (64, 2, 512, 512) fp32
    x0 = x[:, 0]                               (B=64, V=512, F=512)
    n  = sqrt(sum(x0^2, axis=(0, 2)))          (V,)
    out[b] = (x0[b] @ x0[b].T) / outer(n, n)   (B, V, V)

Since gram[b,i,j]/(n_i n_j) == (x0[b,i,:]/n_i) . (x0[b,j,:]/n_j), the host
prescales rows by 1/n once and the device work is a pure batched symmetric
matmul out[b] = y[b] @ y[b].T.

Device-side tricks:
  * operands shipped as fp16 (|y| <= ~0.05, comfortably normal) — halves
    input DMA, full-rate PE, fp32 PSUM accumulation keeps rel err ~2e-4.
  * out[b] is symmetric, and the reference is *exactly* symmetric (same
    products, same summation order), so the device computes only the upper
    block-triangle (row-block mi covers columns mi*128..511) and the host
    mirrors the lower blocks.  -37.5% output DMA, -37.5% PE work.
  * output shipped as a packed fp16 [128, 1280] tile per batch (one DMA,
    2560B contiguous rows) — halves output HBM traffic again and cuts
    descriptor-generation work 4x; host unpacks + upcasts.
  * input shipped pair-of-batches interleaved so each DMA row is 2KB
    (better SDMA descriptor efficiency than 1KB rows).

Sharding: data-parallel over batch — 8 batches per core, no collectives.
"""

import numpy as np

B, T, V, F = 64, 2, 512, 512
NCORES = 8
BPC = B // NCORES  # batches per core
NBLK = V // 128  # 4 row-blocks
NG = BPC // 2  # pair-groups per core

# upper-triangle segment offsets inside the per-batch packed output tile:
# row-block mi holds columns mi*128..511 (n_cols = 512 - 128*mi)
SEG_OFF = [0]
for _mi in range(NBLK):
    SEG_OFF.append(SEG_OFF[-1] + V - 128 * _mi)
SEG_TOTAL = SEG_OFF[-1]  # 1280

_NC = None


def _build_nc():
    import concourse.mybir as mybir
    import concourse.tile as tile
    from concourse import bacc

    f32 = mybir.dt.float32
    f16 = mybir.dt.float16

    nc = bacc.Bacc(target_bir_lowering=False)
    # yP[g, f, b*V + v] = y[2g+b, v, f]  — 2KB rows per partition line
    yP = nc.declare_dram_parameter("yP", [NG, F, 2 * V], f16, isOutput=False)
    outP = nc.declare_dram_parameter(
        "outP", [BPC, 128, SEG_TOTAL], f16, isOutput=True
    )

    with tile.TileContext(nc) as tc:
        with (
            tc.tile_pool(name="sb", bufs=24) as sb_pool,
            tc.tile_pool(name="psum", bufs=8, space="PSUM") as psum_pool,
        ):
            # PE warmup: the HAM clock gate keeps the PE at 1.2 GHz until it
            # has seen ~3.4us of sustained activity.  Burn that window on
            # garbage matmuls while the first input chunks stream in so the
            # real matmuls run at 2.4 GHz from the start.
            warm = sb_pool.tile([128, 640], f16, tag="warm")
            nc.gpsimd.memset(warm, 0)
            wps = psum_pool.tile([128, 512], f32, tag="ps")
            for _ in range(6):
                nc.tensor.matmul(
                    wps,
                    lhsT=warm[:, 512:640],
                    rhs=warm[:, 0:512],
                    start=True,
                    stop=True,
                )
            for g in range(NG):
                # 4 partition-chunks of [128, 2V]: two batches side by side.
                # For the first group, alternate chunks across the two HWDGE
                # rings (SP + ACT) so chunk0 drains alone on its ring and the
                # PE can start ~2us earlier; later groups all ride SP (the
                # ACT ring carries output DMAs by then).
                chunks = []
                for ki in range(NBLK):
                    ck = sb_pool.tile([128, 2 * V], f16, tag="ck")
                    eng = nc.scalar if (g == 0 and ki % 2 == 1) else nc.sync
                    eng.dma_start(
                        out=ck, in_=yP[g, ki * 128 : (ki + 1) * 128, :]
                    )
                    chunks.append(ck)
                for b in range(2):
                    bb = 2 * g + b
                    base = b * V
                    ot = sb_pool.tile([128, SEG_TOTAL], f16, tag="ot")
                    pss = [
                        psum_pool.tile(
                            [128, V - 128 * mi], f32, tag="ps",
                            name=f"ps_{bb}_{mi}",
                        )
                        for mi in range(NBLK)
                    ]
                    if g == 0:
                        # ki-outer: accumulate all 4 row-blocks per input
                        # chunk so the PE starts as soon as chunk0 lands
                        # instead of waiting for the whole group.
                        order = [
                            (ki, mi)
                            for ki in range(NBLK)
                            for mi in range(NBLK)
                        ]
                    else:
                        # mi-outer: each row-block finishes (and casts/ships)
                        # while the next one computes — keeps the tail short.
                        order = [
                            (ki, mi)
                            for mi in range(NBLK)
                            for ki in range(NBLK)
                        ]
                    for ki, mi in order:
                        nc.tensor.matmul(
                            pss[mi],
                            lhsT=chunks[ki][
                                :, base + mi * 128 : base + (mi + 1) * 128
                            ],
                            rhs=chunks[ki][:, base + mi * 128 : base + V],
                            start=(ki == 0),
                            stop=(ki == NBLK - 1),
                        )
                    for mi in range(NBLK):
                        nc.vector.tensor_copy(
                            out=ot[:, SEG_OFF[mi] : SEG_OFF[mi + 1]],
                            in_=pss[mi],
                        )
                    # split the output DMA so earlier blocks ship while later
                    # ones are still being cast; the last batch ships finer,
                    # with its final sliver on the idle SP ring so the two
                    # last triggers run concurrently.
                    if bb == BPC - 1:
                        nc.scalar.dma_start(
                            out=outP[bb][:, 0 : SEG_OFF[2]],
                            in_=ot[:, 0 : SEG_OFF[2]],
                        )
                        nc.scalar.dma_start(
                            out=outP[bb][:, SEG_OFF[2] : SEG_OFF[3]],
                            in_=ot[:, SEG_OFF[2] : SEG_OFF[3]],
                        )
                        nc.sync.dma_start(
                            out=outP[bb][:, SEG_OFF[3] :],
                            in_=ot[:, SEG_OFF[3] :],
                        )
                    else:
                        nc.scalar.dma_start(
                            out=outP[bb][:, 0 : SEG_OFF[2]],
                            in_=ot[:, 0 : SEG_OFF[2]],
                        )
                        nc.scalar.dma_start(
                            out=outP[bb][:, SEG_OFF[2] :],
                            in_=ot[:, SEG_OFF[2] :],
                        )
    if not nc.is_finalized():
        nc.finalize()
    return nc


def _get_nc():
    global _NC
    if _NC is None:
        _NC = _build_nc()
    return _NC


def _prep_shards(x: np.ndarray) -> np.ndarray:
    x = np.ascontiguousarray(np.asarray(x, dtype=np.float32))
    x0 = x[:, 0]  # (B, V, F)
    ss = np.einsum("bvf,bvf->v", x0, x0, optimize=True)
    inv_n = (1.0 / np.sqrt(ss)).astype(np.float32)
    y = x0 * inv_n[None, :, None]
    # (NCORES, NG, F, 2V): per core, pair-groups of batches, F-major so each
    # DMA partition row is a contiguous 2KB line covering both batches.
    yP = np.transpose(
        y.reshape(NCORES, NG, 2, V, F), (0, 1, 4, 2, 3)
    ).reshape(NCORES, NG, F, 2 * V)
    return np.ascontiguousarray(yP.astype(np.float16))


def kernel(x: np.ndarray, _trace: bool = False, _trace_out: list | None = None):
    from concourse.bass_utils import run_bass_kernel_spmd

    yP = _prep_shards(x)
    nc = _get_nc()
    in_maps = [{"yP": yP[c]} for c in range(NCORES)]
    res = run_bass_kernel_spmd(
        nc, in_maps, core_ids=list(range(NCORES)), trace=_trace
    )
    if _trace_out is not None:
        _trace_out.append(res)
    packed = np.concatenate(
        [np.asarray(res.results[c]["outP"]) for c in range(NCORES)], axis=0
    )  # (B, 128, 1280) fp16
    full = np.empty((B, V, V), dtype=np.float32)
    for mi in range(NBLK):
        n_cols = V - 128 * mi
        full[:, mi * 128 : (mi + 1) * 128, mi * 128 :] = packed[
            :, :, SEG_OFF[mi] : SEG_OFF[mi] + n_cols
        ].astype(np.float32)
    # device wrote only the upper block-triangle; mirror it down
    for mi in range(NBLK):
        for nj in range(mi + 1, NBLK):
            full[:, nj * 128 : (nj + 1) * 128, mi * 128 : (mi + 1) * 128] = (
                np.swapaxes(
                    full[:, mi * 128 : (mi + 1) * 128, nj * 128 : (nj + 1) * 128],
                    1,
                    2,
                )
            )
    return full
